# revision 1
# baseline (speedup 1.0000x reference)
"""Trainium2 Bass kernel for per-node multi-head attention (GNN message passing).

Math (per node n):
  q = (h @ Wq + bq).reshape(4, 64);  k, v likewise
  attn = softmax((q @ k.T) / 8, axis=-1)      # [4, 4], across heads
  out  = (attn @ v).reshape(256)

Strategy: pure data parallel over 8 cores (62500 nodes each), node-on-partition
layout (128 nodes per tile).  Per tile:
  PE  : transpose h (bf16), QKV projections (bf16, biases via K=1 ones-row
        matmuls, softmax scale pre-folded into Wq/bq, Wv pre-reordered to
        (d, g) column order so AV products are step-1 innermost)
  ACT : f32->bf16 casts / PSUM->SBUF copies, exp
  DVE : QK pair products (2x bf16), reduce over d, softmax denom/recip/div,
        AV products (2x bf16), add-tree over g -> f32 output
"""

import sys

sys.path.insert(0, "/opt/trn_rl_repo")

import numpy as np
import ml_dtypes

import concourse.bass as bass
import concourse.bacc as bacc
import concourse.tile as tile
from concourse import mybir
from concourse.bass_utils import run_bass_kernel_spmd
from concourse.masks import make_identity

N_CORES = 8
N_TOTAL = 500000
SHARD = N_TOTAL // N_CORES  # 62500
IN = 256
OUT = 256
NH = 4
HD = 64
P = 128

BF16 = mybir.dt.bfloat16
F32 = mybir.dt.float32
ALU = mybir.AluOpType
AX = mybir.AxisListType
ACTF = mybir.ActivationFunctionType


def build_program(shard_rows: int, compile: bool = True) -> bass.Bass:
    # Bacc (not raw Bass): its compile() runs move_matmul_waits_to_ldweights
    # + generate_event_semaphores, which legalize sync waits to the TRN2
    # per-instruction limits (1 wait; EventSemaphore holds 2).
    nc = bacc.Bacc()

    h_ext = nc.declare_dram_parameter("h", [shard_rows, IN], F32, isOutput=False)
    wq_ext = nc.declare_dram_parameter("wq", [IN, OUT], BF16, isOutput=False)
    wk_ext = nc.declare_dram_parameter("wk", [IN, OUT], BF16, isOutput=False)
    wv_ext = nc.declare_dram_parameter("wv", [IN, OUT], BF16, isOutput=False)
    bias_ext = nc.declare_dram_parameter("bias", [3, OUT], BF16, isOutput=False)
    out_ext = nc.declare_dram_parameter("out", [shard_rows, OUT], F32, isOutput=True)

    n_full, tail = divmod(shard_rows, P)
    tiles = [(i, P) for i in range(n_full)]
    if tail:
        tiles.append((n_full, tail))

    with tile.TileContext(nc) as tc:
        with (
            tc.tile_pool(name="consts", bufs=1) as consts,
            tc.tile_pool(name="io", bufs=8) as io,
            tc.tile_pool(name="work", bufs=3) as work,
            tc.tile_pool(name="ps", bufs=2, space="PSUM") as ps,
        ):
            ident = consts.tile([P, P], F32)
            make_identity(nc, ident)

            # Moving operands for the projections: [Kchunk partition, chunk, proj, col]
            # Const DMAs go on the ACT HWDGE ring so the per-tile stream on
            # the SP ring never accumulates cross-lane waits on them (the
            # DIRECT2D DMA instruction supports at most 2 sync waits).
            w_sb = consts.tile([P, 2, 3, OUT], BF16)
            for c in range(2):
                for j, w in enumerate((wq_ext, wk_ext, wv_ext)):
                    nc.scalar.dma_start(
                        out=w_sb[:, c, j], in_=w[c * P : (c + 1) * P, :]
                    )
            bias_sb = consts.tile([1, 3, OUT], BF16)
            nc.scalar.dma_start(out=bias_sb[0:1], in_=bias_ext[:, :])
            ones_sb = consts.tile([1, P], BF16)
            nc.vector.memset(ones_sb, 1.0)

            for i, p in tiles:
                r0 = i * P
                hf = io.tile([p, IN], F32, tag="hf")
                nc.sync.dma_start(out=hf, in_=h_ext[r0 : r0 + p, :])

                # f32 PE transpose straight from the DMA tile (hf has exactly
                # one reader -> the h-in DMA's WAR dep stays a single PE wait);
                # the PSUM->SBUF copy below does the bf16 cast.
                hT = ps.tile([P, 2, p], F32, tag="hT")
                for c in range(2):
                    nc.tensor.transpose(
                        hT[:, c], hf[:, c * P : (c + 1) * P], ident[:p, :p]
                    )
                hTs = work.tile([P, 2, p], BF16, tag="hTs")
                nc.scalar.copy(out=hTs, in_=hT)

                # q+k share one PSUM bank (N=512, one accumulation group);
                # v is its own bank (N=256).
                qkv_ps = ps.tile([p, 3 * OUT], F32, tag="qkv_ps")
                for c in range(2):
                    nc.tensor.matmul(
                        out=qkv_ps[:, 0:512],
                        lhsT=hTs[:, c, :],
                        rhs=w_sb[:, c, 0:2].rearrange("p a b -> p (a b)"),
                        start=(c == 0),
                        stop=False,
                    )
                    nc.tensor.matmul(
                        out=qkv_ps[:, 512:768],
                        lhsT=hTs[:, c, :],
                        rhs=w_sb[:, c, 2],
                        start=(c == 0),
                        stop=False,
                    )
                nc.tensor.matmul(
                    out=qkv_ps[:, 0:512],
                    lhsT=ones_sb[:, :p],
                    rhs=bias_sb[:, 0:2].rearrange("p a b -> p (a b)"),
                    start=False,
                    stop=True,
                )
                nc.tensor.matmul(
                    out=qkv_ps[:, 512:768],
                    lhsT=ones_sb[:, :p],
                    rhs=bias_sb[:, 2],
                    start=False,
                    stop=True,
                )

                qkv = work.tile([p, 3 * OUT], BF16, tag="qkv")
                nc.scalar.copy(out=qkv, in_=qkv_ps)

                # QK products: P1[n, h, g, d] = q[n, h, d] * k[n, g, d]
                p1 = work.tile([p, NH, NH, HD], BF16, tag="p1")
                qb = (
                    qkv[:, 0:256]
                    .rearrange("p (h one d) -> p h one d", h=NH, one=1)
                    .to_broadcast([p, NH, NH, HD])
                )
                kb = (
                    qkv[:, 256:512]
                    .rearrange("p (one g d) -> p one g d", one=1, g=NH)
                    .to_broadcast([p, NH, NH, HD])
                )
                nc.vector.tensor_tensor(out=p1, in0=qb, in1=kb, op=ALU.mult)

                logits = work.tile([p, NH * NH], F32, tag="logits")
                nc.vector.tensor_reduce(
                    out=logits,
                    in_=p1.rearrange("p h g d -> p (h g) d"),
                    axis=AX.X,
                    op=ALU.add,
                )

                ex = work.tile([p, NH * NH], BF16, tag="ex")
                nc.scalar.activation(out=ex, in_=logits, func=ACTF.Exp)

                den = work.tile([p, NH], F32, tag="den")
                nc.vector.tensor_reduce(
                    out=den,
                    in_=ex.rearrange("p (h g) -> p h g", h=NH),
                    axis=AX.X,
                    op=ALU.add,
                )
                rcp = work.tile([p, NH], F32, tag="rcp")
                nc.vector.reciprocal(out=rcp, in_=den)

                attn = work.tile([p, NH, NH], BF16, tag="attn")
                nc.vector.tensor_tensor(
                    out=attn,
                    in0=ex.rearrange("p (h g) -> p h g", h=NH),
                    in1=rcp.rearrange("p (h one) -> p h one", one=1).to_broadcast(
                        [p, NH, NH]
                    ),
                    op=ALU.mult,
                )

                # AV products: P2[n, h, d, g] = attn[n, h, g] * v[n, d, g]
                # (v was projected with (d, g)-reordered columns)
                p2 = work.tile([p, NH, HD, NH], BF16, tag="p2")
                ab = (
                    attn.rearrange("p h (one g) -> p h one g", one=1)
                    .to_broadcast([p, NH, HD, NH])
                )
                vb = (
                    qkv[:, 512:768]
                    .rearrange("p (one d g) -> p one d g", one=1, d=HD)
                    .to_broadcast([p, NH, HD, NH])
                )
                nc.vector.tensor_tensor(out=p2, in0=ab, in1=vb, op=ALU.mult)

                t1 = work.tile([p, NH, HD, 2], BF16, tag="t1")
                nc.vector.tensor_tensor(
                    out=t1, in0=p2[:, :, :, 0:2], in1=p2[:, :, :, 2:4], op=ALU.add
                )
                osb = io.tile([p, OUT], F32, tag="osb")
                nc.vector.tensor_tensor(
                    out=osb.rearrange("p (h d) -> p h d", h=NH),
                    in0=t1[:, :, :, 0],
                    in1=t1[:, :, :, 1],
                    op=ALU.add,
                )

                nc.sync.dma_start(out=out_ext[r0 : r0 + p, :], in_=osb)

    if compile:
        nc.compile()
    return nc


def prepare_weights(Wq, bq, Wk, bk, Wv, bv):
    """Host-side transforms: fold softmax scale into q, reorder Wv/bv to
    (d, g) column order, cast to bf16."""
    scale = 1.0 / np.sqrt(np.float32(HD))
    bf = ml_dtypes.bfloat16
    wq = (np.asarray(Wq, np.float32) * scale).astype(bf)
    wk = np.asarray(Wk, np.float32).astype(bf)
    cols = np.arange(OUT)
    perm = (cols % HD) * NH + cols // HD  # old col (g*64+d) -> new col (d*4+g)
    wv_r = np.empty((IN, OUT), np.float32)
    wv_r[:, perm] = np.asarray(Wv, np.float32)
    bv_r = np.empty((OUT,), np.float32)
    bv_r[perm] = np.asarray(bv, np.float32)
    bias = np.stack(
        [
            np.asarray(bq, np.float32) * scale,
            np.asarray(bk, np.float32),
            bv_r,
        ]
    ).astype(bf)
    return wq, wk, wv_r.astype(bf), bias


_PROGRAM_CACHE = {}


def _get_program(rows):
    if rows not in _PROGRAM_CACHE:
        _PROGRAM_CACHE[rows] = build_program(rows)
    return _PROGRAM_CACHE[rows]


def kernel(h, Wk, bk, Wq, bq, Wv, bv):
    h = np.ascontiguousarray(np.asarray(h, dtype=np.float32))
    wq, wk, wv, bias = prepare_weights(Wq, bq, Wk, bk, Wv, bv)

    nc = _get_program(SHARD)
    in_maps = []
    for i in range(N_CORES):
        in_maps.append(
            {
                "h": h[i * SHARD : (i + 1) * SHARD],
                "wq": wq,
                "wk": wk,
                "wv": wv,
                "bias": bias,
            }
        )
    res = run_bass_kernel_spmd(nc, in_maps, core_ids=list(range(N_CORES)))
    return np.concatenate([res.results[i]["out"] for i in range(N_CORES)], axis=0)

